# revision 10
# baseline (speedup 1.0000x reference)
"""Butterworth 6-section cascaded biquad IIR over [32, 64, 16384], Trainium2 Bass kernel.

Algorithm: exact block state-space decomposition of the linear recurrence.
Split each length-16384 sequence into 128 chunks of B=128 samples.  For chunk c
with incoming 12-dim cascade state s_c:

    y_c = L @ x_c + P @ s_c          (L: [128,128] lower-triangular Toeplitz of the
                                      impulse response, P[j,:] = C A^j)
    s_{c+1} = A^B s_c + G @ x_c      (G[:,k] = A^{B-1-k} B_vec)

Chunk states are propagated in bundles of K=7 chunks with a single [128,16]
matmul per bundle (stacked [s_c; G x_c; ...; G x_{c+6}] -> s_{c+7}), and all 7
intra-bundle states come from one [128,112] matmul on the same stacked tile.
Everything is TensorEngine matmuls; sharding is pure data parallel over 8 cores
(256 sequences per core).
"""

import numpy as np

B = 128          # chunk length
C = 128          # chunks per sequence (T = 16384)
T = C * B
KB = 3           # bundle size: 4 blocks x 32-partition stride = 128 contraction rows
NSEQ = 256       # sequences per core
N_CORES = 8
SD = 16          # padded state dim (real state dim = 12)
BLK = 32         # partition stride of stacked blocks (engine partition-start rule)

_BUNDLES = [(c0, min(KB, C - c0)) for c0 in range(0, C, KB)]  # 18 full + (126,2)

_COMPILED = None  # (nc, const_names)


def _build_matrices(sos):
    """Float64 host precompute of all device constant matrices from sos [6,6]."""
    sos = np.asarray(sos, dtype=np.float64)
    S = sos.shape[0]
    b0, b1, b2 = sos[:, 0], sos[:, 1], sos[:, 2]
    a1, a2 = sos[:, 4], sos[:, 5]
    n = 2 * S  # 12

    def step(s, u):
        s = s.copy()
        cur = u
        for i in range(S):
            w1, w2 = s[2 * i], s[2 * i + 1]
            y = b0[i] * cur + w1
            s[2 * i] = b1[i] * cur - a1[i] * y + w2
            s[2 * i + 1] = b2[i] * cur - a2[i] * y
            cur = y
        return s, cur

    A = np.zeros((n, n))
    Cv = np.zeros(n)
    for j in range(n):
        e = np.zeros(n)
        e[j] = 1.0
        A[:, j], Cv[j] = step(e, 0.0)
    Bv, D = step(np.zeros(n), 1.0)

    Apow = [np.eye(n)]
    for _ in range(B):
        Apow.append(Apow[-1] @ A)

    h = np.zeros(B)
    h[0] = D
    for m in range(1, B):
        h[m] = Cv @ Apow[m - 1] @ Bv
    L = np.zeros((B, B))
    for j in range(B):
        L[j, : j + 1] = h[j::-1]

    G = np.stack([Apow[B - 1 - k] @ Bv for k in range(B)], axis=1)   # [12, B]
    P = np.stack([Cv @ Apow[j] for j in range(B)], axis=0)           # [B, 12]
    ABp = [np.eye(n)]
    for _ in range(KB):
        ABp.append(ABp[-1] @ Apow[B])

    f32 = np.float32
    lt = L.T.astype(f32).copy()                                       # [128,128]
    g16t = np.zeros((B, SD), f32)
    g16t[:, :n] = G.T                                                 # [128,16]
    pt = np.zeros((SD, B), f32)
    pt[:n, :] = P.T                                                   # [16,128]
    # stacked rows (stride BLK): [s_c | S0_c | S0_{c+1} | S0_{c+2}]
    # chain: stacked -> s_{c+KB}
    ch = np.zeros(((KB + 1) * BLK, SD), f32)                          # [128,16]
    for e in range(KB + 1):
        ch[e * BLK : e * BLK + n, :n] = ABp[KB - e].T
    # all intra-bundle states: out block d (stride BLK) gets A_B^{d-e} on block e
    sall = np.zeros(((KB + 1) * BLK, KB * BLK), f32)                  # [128,96]
    for d in range(KB):
        for e in range(d + 1):
            sall[e * BLK : e * BLK + n, d * BLK : d * BLK + n] = ABp[d - e].T
    ident = np.eye(B, dtype=f32)
    return dict(lt=lt, g16t=g16t, pt=pt, ch=ch, sall=sall, ident=ident)


def _build_program():
    import concourse.mybir as mybir
    from concourse import bacc
    from concourse.tile import TileContext

    f32 = mybir.dt.float32
    nc = bacc.Bacc(trn_type="TRN2")

    x = nc.dram_tensor("x", [NSEQ, T], f32, kind="ExternalInput")
    y = nc.dram_tensor("y", [NSEQ, T], f32, kind="ExternalOutput")
    lt_d = nc.dram_tensor("lt", [B, B], f32, kind="ExternalInput")
    g16t_d = nc.dram_tensor("g16t", [B, SD], f32, kind="ExternalInput")
    pt_d = nc.dram_tensor("pt", [SD, B], f32, kind="ExternalInput")
    ch_d = nc.dram_tensor("ch", [(KB + 1) * BLK, SD], f32, kind="ExternalInput")
    sall_d = nc.dram_tensor("sall", [(KB + 1) * BLK, KB * BLK], f32, kind="ExternalInput")
    ident_d = nc.dram_tensor("ident", [B, B], f32, kind="ExternalInput")

    NB = len(_BUNDLES)
    GRP = 16                      # chunks per DMA group (2048 columns)
    copy_ctr = [0]

    def big_copy(dst, src):
        # All PSUM->SBUF copies on DVE: a tile written by multiple engines
        # forces multi-semaphore waits on its consumers, and matmul (LDW)
        # instructions have a tight HW wait-command budget.
        nc.vector.tensor_copy(dst, src)
        copy_ctr[0] += 1

    with TileContext(nc) as tc:
        with (
            tc.tile_pool(name="consts", bufs=1) as cpool,
            tc.tile_pool(name="xin", bufs=2) as xin_pool,
            tc.tile_pool(name="xp", bufs=16) as xp_pool,
            tc.tile_pool(name="st", bufs=16) as st_pool,
            tc.tile_pool(name="ys", bufs=2) as ys_pool,
            tc.tile_pool(name="tp_ps", bufs=2, space="PSUM") as tp_ps,
            tc.tile_pool(name="gp_ps", bufs=2, space="PSUM") as gp_ps,
            tc.tile_pool(name="stp_ps", bufs=1, space="PSUM") as stp_ps,
            tc.tile_pool(name="cp_ps", bufs=1, space="PSUM") as cp_ps,
            tc.tile_pool(name="yp_ps", bufs=2, space="PSUM") as yp_ps,
        ):
            lt = cpool.tile_from(lt_d[:, :], name="lt_sb")
            g16t = cpool.tile_from(g16t_d[:, :], name="g16t_sb")
            pt = cpool.tile_from(pt_d[:, :], name="pt_sb")
            ch = cpool.tile_from(ch_d[:, :], name="ch_sb")
            sall = cpool.tile_from(sall_d[:, :], name="sall_sb")
            ident = cpool.tile_from(ident_d[:, :], name="ident_sb")
            # per-bundle stacked [s ; S0 x KB] tiles, all in one long-lived tile
            stacked = cpool.tile([(KB + 1) * BLK, NB * NSEQ], f32, name="stacked")

            # zero everything once: s_0 = 0 and the unused 16-row pads of each
            # 32-row block must not contain NaN garbage (they're contracted
            # against zero coefficients, but 0*NaN = NaN).
            nc.vector.memset(stacked[:, :], 0.0)

            # Preamble: touch every constant from the PE with single-dependency
            # dummy ops.  Matmult carries at most ONE semaphore wait in codegen,
            # so each constant's DMA-lane semaphore must enter PE's observed
            # clock before any real matmul needs the constant plus another
            # operand.  (Outputs are garbage and never read.)
            pre = tp_ps.tile([B, B], f32, tag="tp", name="pre")
            nc.tensor.transpose(pre, ident, ident)
            nc.tensor.matmul(pre, lt, ident, start=True, stop=True)
            nc.tensor.matmul(pre, pt, pt, start=True, stop=True)
            nc.tensor.matmul(pre[0:SD, :], g16t, ident, start=True, stop=True)
            nc.tensor.matmul(pre[0:SD, :], ch, ident, start=True, stop=True)
            nc.tensor.matmul(pre[0:KB * BLK, :], sall, ident, start=True, stop=True)

            xin = {}      # sb -> current input group tile
            ystage = {}   # sb -> current output staging tile
            xpan = {}     # c -> x-columns panel [128 k, 256 seq]

            for b, (c0, kb) in enumerate(_BUNDLES):
                bcols = slice(b * NSEQ, (b + 1) * NSEQ)
                # ---- transpose-in + G state injection ----
                for d in range(kb):
                    c = c0 + d
                    g, cl = divmod(c, GRP)
                    if cl == 0:
                        for sb in (0, 1):
                            xt = xin_pool.tile([B, GRP * B], f32, tag=f"xin{sb}",
                                               name=f"xin{sb}_{g}")
                            nc.sync.dma_start(
                                xt, x[sb * B : (sb + 1) * B, g * GRP * B : (g + 1) * GRP * B])
                            xin[sb] = xt
                    xp = xp_pool.tile([B, NSEQ], f32, tag="xp", name=f"xp_{c}")
                    for sb in (0, 1):
                        tp = tp_ps.tile([B, B], f32, tag="tp", name=f"tp_{c}_{sb}")
                        nc.tensor.transpose(tp, xin[sb][:, cl * B : (cl + 1) * B], ident)
                        big_copy(xp[:, sb * B : (sb + 1) * B], tp)
                    xpan[c] = xp
                    gp = gp_ps.tile([SD, NSEQ], f32, tag="gp", name=f"gp_{c}")
                    nc.tensor.matmul(gp, g16t, xp, start=True, stop=True)
                    nc.vector.tensor_copy(
                        stacked[BLK * (d + 1) : BLK * (d + 1) + SD, bcols], gp)
                # ---- intra-bundle states ----
                rows, mcols = BLK * (kb + 1), BLK * kb
                stp = stp_ps.tile([KB * BLK, NSEQ], f32, tag="stp", name=f"stp_{b}")
                nc.tensor.matmul(stp[0:mcols, :], sall[0:rows, 0:mcols],
                                 stacked[0:rows, bcols], start=True, stop=True)
                sts = []
                for d in range(kb):
                    st = st_pool.tile([SD, NSEQ], f32, tag="st", name=f"st_{b}_{d}")
                    nc.vector.tensor_copy(st, stp[BLK * d : BLK * d + SD, :])
                    sts.append(st)
                # ---- chain to next bundle ----
                if b < NB - 1:
                    cp = cp_ps.tile([SD, NSEQ], f32, tag="cp", name=f"cp_{b}")
                    nc.tensor.matmul(cp, ch, stacked[:, bcols], start=True, stop=True)
                    nc.vector.tensor_copy(
                        stacked[0:SD, (b + 1) * NSEQ : (b + 2) * NSEQ], cp)
                # ---- Y = L @ x_c + P @ s_c, natural [seq, time] layout ----
                for d in range(kb):
                    c = c0 + d
                    g, cl = divmod(c, GRP)
                    for sb in (0, 1):
                        if cl == 0 or sb not in ystage:
                            ystage[sb] = ys_pool.tile([B, GRP * B], f32, tag=f"ys{sb}",
                                                      name=f"ys{sb}_{g}")
                        yp = yp_ps.tile([B, B], f32, tag="yp", name=f"yp_{c}_{sb}")
                        nc.tensor.matmul(yp, xpan[c][:, sb * B : (sb + 1) * B], lt,
                                         start=True, stop=False)
                        nc.tensor.matmul(yp, sts[d][:, sb * B : (sb + 1) * B], pt,
                                         start=False, stop=True)
                        big_copy(ystage[sb][:, cl * B : (cl + 1) * B], yp)
                        if cl == GRP - 1:
                            nc.sync.dma_start(
                                y[sb * B : (sb + 1) * B, g * GRP * B : (g + 1) * GRP * B],
                                ystage[sb])
                    if cl == GRP - 1:
                        ystage = {}
                    del xpan[c]
    nc.compile()  # bacc passes: register alloc + matmul wait splitting
    return nc


def _get_compiled():
    global _COMPILED
    if _COMPILED is None:
        _COMPILED = _build_program()
    return _COMPILED


def kernel(x, sos):
    x = np.asarray(x, dtype=np.float32)
    sos = np.asarray(sos)
    orig_shape = x.shape
    x2 = np.ascontiguousarray(x.reshape(-1, T))          # [2048, 16384]
    per = x2.shape[0] // N_CORES                          # 256

    consts = _build_matrices(sos)
    nc = _get_compiled()

    in_maps = []
    for cid in range(N_CORES):
        m = {"x": np.ascontiguousarray(x2[cid * per : (cid + 1) * per])}
        m.update(consts)
        in_maps.append(m)

    from concourse.bass_utils import run_bass_kernel_spmd
    res = run_bass_kernel_spmd(nc, in_maps, core_ids=list(range(N_CORES)))
    y2 = np.concatenate([r["y"] for r in res.results], axis=0)
    return y2.reshape(orig_shape).astype(np.float32)
